# revision 1
# baseline (speedup 1.0000x reference)
"""Trainium2 Bass kernel for the looped ("fractal") transformer LM.

Hardcoded shapes: B=1, T=1024, D=1024, H=16 heads (HD=64), FF=4096, V=32000,
L=24 layers sharing one block's weights, G=16 rotation groups (g=64).

Strategy: sequence-parallel over 8 NeuronCores (128 tokens each).
 - Activations transposed on-chip: x^T is [128(d-in-tile), 8(d-tile), 128(t)].
 - Per layer each core computes Q/K/V for its tokens; K and V_aug (V with a
   ones column per head for the softmax denominator) are all-gathered in bf16.
 - Scores computed transposed ([kt, qt]); softmax without max-subtraction
   (scores are O(1) after rmsnorm); causality/cross-core zeroing via a 0/1
   mask multiply on exp(s); denominator rides in V_aug's ones column.
 - Matmuls bf16 with fp32 PSUM; residual stream / norms / rotations fp32.
 - Norm weights folded host-side (ln1->gamma, ln2->w1, final norm->lm_head,
   beta->qkv bias). lm_head vocab-sharded, bf16 stationaries.
"""

import numpy as np
import ml_dtypes

import concourse.bass as bass
import concourse.mybir as mybir
import concourse.tile as tile
from concourse import bacc
from concourse.bass_utils import run_bass_kernel_spmd

F32 = mybir.dt.float32
BF16 = mybir.dt.bfloat16
ADD = mybir.AluOpType.add
MULT = mybir.AluOpType.mult
AF = mybir.ActivationFunctionType

D = 1024
H = 16
HD = 64
V = 32000
L = 24
G = 16
FF = 4 * D
T = 1024
NC = 8
TC = T // NC           # 128
DT = D // 128          # 8
FFT = FF // 128        # 32
VC = V // NC           # 4000
VAW = D + H            # 1040 (V_aug width: 16 heads x 65)
KVW = D + VAW          # 2064
EPS = 1e-6


def _cayley_rotations(skew_upper):
    rows, cols = np.triu_indices(64, k=1)
    su = np.asarray(skew_upper, np.float64)
    A = np.zeros((L, G, 64, 64), np.float64)
    A[:, :, rows, cols] = su
    A = A - np.swapaxes(A, -1, -2)
    I = np.eye(64)
    return np.linalg.solve(I[None, None] - 0.5 * A, I[None, None] + 0.5 * A)


def _build_program(s_vals, n_layers=L, stage=99):
    nc = bacc.Bacc("TRN2", target_bir_lowering=False, debug=False, num_devices=NC)

    x0_in = nc.dram_tensor("x0", [128, DT, TC], F32, kind="ExternalInput")
    wqk_in = nc.dram_tensor("wqk", [DT, 128, 2 * D], BF16, kind="ExternalInput")
    wv_in = nc.dram_tensor("wv", [DT, 128, D], BF16, kind="ExternalInput")
    # wo: 2 chunks x [128, 4*1024]; w1/w2: 8 chunks x [128, 4096]
    wo_in = nc.dram_tensor("wo", [2, 128, 4 * D], BF16, kind="ExternalInput")
    w1_in = nc.dram_tensor("w1", [8, 128, 4 * D], BF16, kind="ExternalInput")
    w2_in = nc.dram_tensor("w2", [8, 128, 4 * D], BF16, kind="ExternalInput")
    rp_in = nc.dram_tensor("rp", [L, 128, DT * 128], F32, kind="ExternalInput")
    gam_in = nc.dram_tensor("gam", [128, DT, L], F32, kind="ExternalInput")
    bet_in = nc.dram_tensor("bet", [128, DT, L], F32, kind="ExternalInput")
    mask_in = nc.dram_tensor("mask", [128, NC, TC], BF16, kind="ExternalInput")
    lmh_in = nc.dram_tensor("lmh", [DT, 128, VC], BF16, kind="ExternalInput")
    ones_in = nc.dram_tensor("onesv", [128, 1], F32, kind="ExternalInput")
    logits_out = nc.dram_tensor("logits", [VC, T], F32, kind="ExternalOutput")
    xf_out = nc.dram_tensor("xfinal", [128, DT, TC], F32, kind="ExternalOutput")

    kv_all = nc.dram_tensor("kv_all", [NC, 128, KVW], BF16, addr_space="Shared")
    xa_all = nc.dram_tensor("xa_all", [NC, 128, DT, TC], BF16, addr_space="Shared")

    with tile.TileContext(nc) as tc:
        with (
            tc.tile_pool(name="resw", bufs=1) as resw,
            tc.tile_pool(name="small", bufs=4) as small,
            tc.tile_pool(name="psA", bufs=4, space="PSUM") as psA,
            tc.tile_pool(name="psB", bufs=2, space="PSUM") as psB,
            tc.tile_pool(name="psS", bufs=2, space="PSUM") as psS,
            tc.tile_pool(name="dram", bufs=3, space="DRAM") as dram,
        ):
            # ---- resident weights / constants ----
            wqk_sb = resw.tile([128, DT, 2 * D], BF16)
            for k in range(DT):
                nc.sync.dma_start(out=wqk_sb[:, k, :], in_=wqk_in[k])
            wv_sb = resw.tile([128, DT, D], BF16)
            for k in range(DT):
                nc.sync.dma_start(out=wv_sb[:, k, :], in_=wv_in[k])
            gam_sb = resw.tile([128, DT, L], F32)
            nc.sync.dma_start(out=gam_sb[:], in_=gam_in[:])
            bet_sb = resw.tile([128, DT, L], F32)
            nc.sync.dma_start(out=bet_sb[:], in_=bet_in[:])
            mask_sb = resw.tile([128, NC, TC], BF16)
            nc.sync.dma_start(out=mask_sb[:], in_=mask_in[:])
            ones_sb = resw.tile([128, 1], F32)
            nc.sync.dma_start(out=ones_sb[:], in_=ones_in[:])
            ones_col = ones_sb[:, 0:1]
            onesr_sb = resw.tile([1, TC], F32)
            nc.vector.memset(onesr_sb[:], 1.0)
            eps_sb = resw.tile([1, 1], F32)
            nc.vector.memset(eps_sb[:], EPS)

            def rmsnorm_rstd(src, tag):
                """src: [128, DT, TC] fp32 -> psum [128, TC] bcast of 1/rms."""
                ss = psB.tile([1, TC], F32, tag="b")
                for k in range(DT):
                    sq = small.tile([128, TC], F32, tag="sq")
                    nc.scalar.activation(sq[:], src[:, k, :], AF.Square)
                    nc.tensor.matmul(ss[:], ones_col, sq[:],
                                     start=(k == 0), stop=(k == DT - 1))
                sd = small.tile([1, TC], F32, tag="sd")
                nc.scalar.activation(sd[:], ss[:], AF.Sqrt, bias=eps_sb[:],
                                     scale=1.0 / D)
                rs = small.tile([1, TC], F32, tag="rs")
                nc.vector.reciprocal(rs[:], sd[:])
                rsb = psB.tile([128, TC], F32, tag="b")
                nc.tensor.matmul(rsb[:], onesr_sb[0:1, 0:128], rs[:],
                                 start=True, stop=True)
                return rsb

            with (
                tc.tile_pool(name="wstr", bufs=1) as wstr,
                tc.tile_pool(name="acts", bufs=2) as acts,
                tc.tile_pool(name="xpool", bufs=2) as xpool,
                tc.tile_pool(name="kvk", bufs=8) as kvk,
                tc.tile_pool(name="kvv", bufs=3) as kvv,
                tc.tile_pool(name="expp", bufs=9) as expp,
            ):
                x_sb = xpool.tile([128, DT, TC], F32, tag="x")
                nc.sync.dma_start(out=x_sb[:], in_=x0_in[:])

                for l in range(n_layers):
                    s_l = float(s_vals[l])
                    # ---- rmsnorm1 (gamma folded; beta via qkv bias) ----
                    rsb1 = rmsnorm_rstd(x_sb, "n1")
                    h_sb = acts.tile([128, DT, TC], BF16, tag="h", bufs=1)
                    for k in range(DT):
                        xh1 = small.tile([128, TC], F32, tag="xh1", bufs=2)
                        nc.vector.tensor_tensor(xh1[:], x_sb[:, k, :], rsb1[:],
                                                op=MULT)
                        nc.vector.tensor_scalar(
                            h_sb[:, k, :], xh1[:], gam_sb[:, k, l:l + 1],
                            bet_sb[:, k, l:l + 1], op0=MULT, op1=ADD)

                    if stage < 2:
                        continue
                    # ---- qkv ----
                    qk_sb = acts.tile([128, 2 * DT, TC], BF16, tag="qk")
                    for jt in range(2 * DT):
                        ps = psA.tile([128, TC], F32, tag="a")
                        for k in range(DT):
                            nc.tensor.matmul(
                                ps[:], wqk_sb[:, k, jt * 128:(jt + 1) * 128],
                                h_sb[:, k, :], start=(k == 0), stop=(k == DT - 1))
                        nc.vector.tensor_copy(qk_sb[:, jt, :], ps[:])
                    va_sb = acts.tile([128, H, 65], BF16, tag="va", bufs=1)
                    for half in range(2):
                        ps = psA.tile([128, 512], F32, tag="a")
                        for k in range(DT):
                            nc.tensor.matmul(
                                ps[:], h_sb[:, k, :],
                                wv_sb[:, k, half * 512:(half + 1) * 512],
                                start=(k == 0), stop=(k == DT - 1))
                        for hh in range(8):
                            head = half * 8 + hh
                            nc.vector.tensor_copy(
                                va_sb[:, head, 0:64], ps[:, hh * 64:(hh + 1) * 64])
                    nc.vector.memset(va_sb[:, :, 64:65], 1.0)

                    if stage < 3:
                        continue
                    # ---- KV allgather ----
                    kv_loc = dram.tile([128, KVW], BF16, tag="kvloc")
                    nc.sync.dma_start(out=kv_loc[:, 0:D], in_=qk_sb[:, DT:2 * DT, :])
                    nc.sync.dma_start(out=kv_loc[:, D:KVW], in_=va_sb[:])
                    nc.gpsimd.collective_compute(
                        "AllGather", mybir.AluOpType.bypass,
                        replica_groups=[list(range(NC))],
                        ins=[kv_loc[:]], outs=[kv_all[:]])

                    k_rt = []
                    v_rt = []
                    for r in range(NC):
                        kt = kvk.tile([128, DT, TC], BF16, tag="krt")
                        nc.sync.dma_start(out=kt[:], in_=kv_all[r, :, 0:D])
                        k_rt.append(kt)
                        vt = kvv.tile([128, H, 65], BF16, tag="vrt", bufs=9)
                        nc.sync.dma_start(out=vt[:], in_=kv_all[r, :, D:KVW])
                        v_rt.append(vt)

                    if stage < 4:
                        continue
                    # ---- attention ----
                    attn_sb = acts.tile([128, DT, TC], BF16, tag="attn", bufs=1)
                    for quad in range(4):
                        exp_t = []
                        for r in range(NC):
                            ex = expp.tile([128, 4, TC], BF16, tag="exp")
                            for hh in range(4):
                                head = quad * 4 + hh
                                po2, pj = (head % 2) * 64, head // 2
                                ps = psS.tile([128, TC], F32, tag="sc")
                                nc.tensor.matmul(
                                    ps[:],
                                    k_rt[r][po2:po2 + 64, pj, :],
                                    qk_sb[po2:po2 + 64, pj, :],
                                    start=True, stop=True)
                                nc.scalar.activation(ex[:, hh, :], ps[:],
                                                     AF.Exp, scale=0.125)
                                nc.vector.tensor_tensor(
                                    ex[:, hh, :], ex[:, hh, :],
                                    mask_sb[:, r, :], op=MULT)
                            exp_t.append(ex)
                        po = [psA.tile([65, TC], F32, tag="a", name=f"po{i}")
                              for i in range(4)]
                        for r in range(NC):
                            vt = v_rt[r]
                            for hh in range(4):
                                head = quad * 4 + hh
                                nc.tensor.matmul(
                                    po[hh][:], vt[:, head, :], exp_t[r][:, hh, :],
                                    start=(r == 0), stop=(r == NC - 1))
                        for hh in range(4):
                            head = quad * 4 + hh
                            dn = small.tile([1, TC], F32, tag="dn")
                            nc.scalar.activation(dn[:], po[hh][64:65, :], AF.Copy)
                            rc = small.tile([1, TC], F32, tag="rc")
                            nc.vector.reciprocal(rc[:], dn[:])
                            pb = psB.tile([64, TC], F32, tag="b")
                            nc.tensor.matmul(pb[:], onesr_sb[0:1, 0:64], rc[:],
                                             start=True, stop=True)
                            pb2 = small.tile([64, TC], F32, tag="pb2", bufs=2)
                            nc.scalar.activation(pb2[:], pb[:], AF.Copy)
                            po2, pj = (head % 2) * 64, head // 2
                            nc.vector.tensor_tensor(
                                attn_sb[po2:po2 + 64, pj, :], po[hh][0:64, :],
                                pb2[:], op=MULT)

                    if stage < 5:
                        continue
                    # ---- wo + residual / blend ----
                    wo_sb = [wstr.tile([128, 4 * D], BF16, tag="wch", bufs=4, name=f"wo{i}")
                             for i in range(2)]
                    for c2 in range(2):
                        nc.sync.dma_start(out=wo_sb[c2][:], in_=wo_in[c2])
                    x1_sb = xpool.tile([128, DT, TC], F32, tag="x1")
                    tb_sb = xpool.tile([128, DT, TC], F32, tag="tb", bufs=1)
                    for m in range(DT):
                        ps = psA.tile([128, TC], F32, tag="a")
                        for k in range(DT):
                            nc.tensor.matmul(
                                ps[:],
                                wo_sb[k // 4][:, (k % 4) * D + m * 128:
                                              (k % 4) * D + (m + 1) * 128],
                                attn_sb[:, k, :], start=(k == 0), stop=(k == DT - 1))
                        nc.vector.tensor_tensor(x1_sb[:, m, :], ps[:], x_sb[:, m, :],
                                                op=ADD)
                        nc.vector.scalar_tensor_tensor(
                            tb_sb[:, m, :], ps[:], s_l, x_sb[:, m, :],
                            op0=MULT, op1=ADD)

                    if stage < 6:
                        continue
                    # ---- rmsnorm2 + FFN (ln2 folded into w1) ----
                    rsb2 = rmsnorm_rstd(x1_sb, "n2")
                    h2_sb = acts.tile([128, DT, TC], BF16, tag="h2", bufs=1)
                    for k in range(DT):
                        nc.vector.tensor_tensor(h2_sb[:, k, :], x1_sb[:, k, :],
                                                rsb2[:], op=MULT)

                    g_sb = acts.tile([128, FFT, TC], BF16, tag="g", bufs=1)
                    for q in range(8):
                        w1c = wstr.tile([128, 4 * D], BF16, tag="wch", bufs=4)
                        nc.sync.dma_start(out=w1c[:], in_=w1_in[q])
                        for j in range(4):
                            ft = 4 * q + j
                            ps = psA.tile([128, TC], F32, tag="a")
                            for k in range(DT):
                                nc.tensor.matmul(
                                    ps[:], w1c[:, (j * DT + k) * 128:
                                               (j * DT + k + 1) * 128],
                                    h2_sb[:, k, :], start=(k == 0),
                                    stop=(k == DT - 1))
                            # gelu(x)*2 = x*(1+tanh(c*(x+0.044715 x^3)));
                            # the 0.5 is folded into w2 on the host.
                            sqg = small.tile([128, TC], F32, tag="sqg", bufs=2)
                            nc.scalar.activation(sqg[:], ps[:], AF.Square)
                            vg = small.tile([128, TC], F32, tag="vg", bufs=2)
                            nc.vector.tensor_scalar(
                                vg[:], sqg[:], 0.044715, 1.0,
                                op0=MULT, op1=ADD)
                            wg = small.tile([128, TC], F32, tag="wg", bufs=2)
                            nc.vector.tensor_tensor(wg[:], vg[:], ps[:], op=MULT)
                            tg = small.tile([128, TC], F32, tag="tg", bufs=2)
                            nc.scalar.activation(tg[:], wg[:], AF.Tanh,
                                                 scale=0.7978845608028654)
                            nc.vector.scalar_tensor_tensor(
                                g_sb[:, ft, :], tg[:], 1.0, ps[:],
                                op0=ADD, op1=MULT)

                    # w2 in 8 single-chunk passes, running partial in SBUF
                    part_sb = acts.tile([128, DT, TC], F32, tag="part", bufs=1)
                    xb_sb = xpool.tile([128, DT, TC], F32, tag="xb", bufs=1)
                    for q in range(8):
                        w2c = wstr.tile([128, 4 * D], BF16, tag="wch", bufs=4)
                        nc.sync.dma_start(out=w2c[:], in_=w2_in[q])
                        for m in range(DT):
                            ps = psA.tile([128, TC], F32, tag="a")
                            for j in range(4):
                                ft = 4 * q + j
                                nc.tensor.matmul(
                                    ps[:], w2c[:, j * D + m * 128:
                                               j * D + (m + 1) * 128],
                                    g_sb[:, ft, :], start=(j == 0), stop=(j == 3))
                            if q == 0:
                                nc.vector.tensor_copy(part_sb[:, m, :], ps[:])
                            elif q < 7:
                                nc.vector.tensor_tensor(part_sb[:, m, :],
                                                        part_sb[:, m, :], ps[:],
                                                        op=ADD)
                            else:
                                nc.vector.tensor_tensor(part_sb[:, m, :],
                                                        part_sb[:, m, :], ps[:],
                                                        op=ADD)
                                nc.vector.scalar_tensor_tensor(
                                    xb_sb[:, m, :], part_sb[:, m, :], s_l,
                                    tb_sb[:, m, :], op0=MULT, op1=ADD)

                    if stage < 7:
                        continue
                    # ---- group rotation (fp32) ----
                    r_sb = wstr.tile([128, DT * 128], F32, tag="rp", bufs=2)
                    nc.sync.dma_start(out=r_sb[:], in_=rp_in[l])
                    x_new = xpool.tile([128, DT, TC], F32, tag="x")
                    for p in range(DT):
                        ps = psA.tile([128, TC], F32, tag="a")
                        nc.tensor.matmul(ps[:], r_sb[:, p * 128:(p + 1) * 128],
                                         xb_sb[:, p, :], start=True, stop=True)
                        nc.scalar.activation(x_new[:, p, :], ps[:], AF.Copy)
                    x_sb = x_new

                # ---- final rmsnorm (norm_w folded into lm_head) ----
                nc.sync.dma_start(out=xf_out[:], in_=x_sb[:])
                rsbf = rmsnorm_rstd(x_sb, "nf")
                xh_sb = acts.tile([128, DT, TC], BF16, tag="h", bufs=1)
                for k in range(DT):
                    nc.vector.tensor_tensor(xh_sb[:, k, :], x_sb[:, k, :],
                                            rsbf[:], op=MULT)
                xh_loc = dram.tile([128, DT, TC], BF16, tag="xhloc")
                nc.sync.dma_start(out=xh_loc[:], in_=xh_sb[:])
                nc.gpsimd.collective_compute(
                    "AllGather", mybir.AluOpType.bypass,
                    replica_groups=[list(range(NC))],
                    ins=[xh_loc[:]], outs=[xa_all[:]])

            # ---- lm_head (layer pools closed; lm pools open) ----
            with (
                tc.tile_pool(name="lmw", bufs=9) as lmw,
                tc.tile_pool(name="lmx", bufs=1) as lmx,
            ):
                xa_sb = lmx.tile([128, DT, T], BF16)
                for r in range(NC):
                    nc.sync.dma_start(out=xa_sb[:, :, r * TC:(r + 1) * TC],
                                      in_=xa_all[r])
                lm_tiles = []
                for k in range(DT):
                    lmk = lmw.tile([128, VC], BF16, tag="lmh")
                    nc.sync.dma_start(out=lmk[:], in_=lmh_in[k])
                    lm_tiles.append(lmk)
                NVT = (VC + 127) // 128
                for tch in range(2):
                    for vt in range(NVT):
                        vsz = min(128, VC - vt * 128)
                        ps = psS.tile([128, 4, TC], F32, tag="sc")
                        pss = ps[0:vsz, :, :]
                        for k in range(DT):
                            nc.tensor.matmul(
                                pss, lm_tiles[k][:, vt * 128:vt * 128 + vsz],
                                xa_sb[:, k, tch * 512:(tch + 1) * 512],
                                start=(k == 0), stop=(k == DT - 1))
                        ot = small.tile([128, 512], F32, tag="lo", bufs=2)
                        nc.vector.tensor_copy(ot[0:vsz, :], pss)
                        nc.sync.dma_start(
                            out=logits_out[vt * 128:vt * 128 + vsz,
                                           tch * 512:(tch + 1) * 512],
                            in_=ot[0:vsz, :])

    nc.finalize()
    return nc


_CACHE = {}


def _prep(inputs):
    tokens = np.asarray(inputs["tokens"])
    f32 = lambda k: np.asarray(inputs[k], np.float32)
    embed_w, lm_head_w, norm_w = f32("embed_w"), f32("lm_head_w"), f32("norm_w")
    layer_gamma, layer_beta = f32("layer_gamma"), f32("layer_beta")
    iter_scale, skew_upper = f32("iter_scale"), f32("skew_upper")
    ln1_w, ln2_w = f32("ln1_w"), f32("ln2_w")
    wqkv, wo, w1, w2 = f32("wqkv"), f32("wo"), f32("w1"), f32("w2")

    s_vals = iter_scale.reshape(-1).astype(np.float64)
    bf = ml_dtypes.bfloat16

    R = _cayley_rotations(skew_upper).astype(np.float32)
    rp = np.zeros((L, 128, DT * 128), np.float32)
    for p in range(DT):
        rp[:, 0:64, p * 128:p * 128 + 64] = R[:, 2 * p]
        rp[:, 64:128, p * 128 + 64:p * 128 + 128] = R[:, 2 * p + 1]

    gamma_eff = layer_gamma.astype(np.float64) * ln1_w.astype(np.float64)
    gam = gamma_eff.reshape(L, DT, 128).transpose(2, 1, 0).astype(np.float32)
    gam = np.ascontiguousarray(gam)
    bet = np.ascontiguousarray(
        layer_beta.reshape(L, DT, 128).transpose(2, 1, 0)).astype(np.float32)

    wqk_h = np.ascontiguousarray(wqkv[:, :2 * D].reshape(DT, 128, 2 * D)).astype(bf)
    wv_h = np.ascontiguousarray(wqkv[:, 2 * D:].reshape(DT, 128, D)).astype(bf)
    wo_h = np.ascontiguousarray(
        wo.reshape(2, 4, 128, D).transpose(0, 2, 1, 3).reshape(2, 128, 4 * D)
    ).astype(bf)
    w1_eff = w1.astype(np.float64) * ln2_w.astype(np.float64)[:, None]
    # w1 chunk q: [128(d-in-tile), (j,k,128)] with ft = 4q+j, d-tile k
    w1_h = np.ascontiguousarray(
        w1_eff.astype(np.float32).reshape(DT, 128, 8, 4, 128)
        .transpose(2, 1, 3, 0, 4).reshape(8, 128, 4 * D)).astype(bf)
    w2_h = np.ascontiguousarray(
        (0.5 * w2).reshape(8, 4, 128, D).transpose(0, 2, 1, 3)
        .reshape(8, 128, 4 * D)).astype(bf)
    lm_eff = (lm_head_w.astype(np.float64) *
              norm_w.astype(np.float64)[None, :]).astype(np.float32)
    x0 = embed_w[tokens.reshape(-1)]
    ones_h = np.ones((128, 1), np.float32)

    in_maps = []
    for c in range(NC):
        x0c = x0[c * TC:(c + 1) * TC].T
        x0h = np.ascontiguousarray(x0c.reshape(DT, 128, TC).transpose(1, 0, 2))
        mask = np.zeros((NC, 128, TC), np.float32)
        for r in range(NC):
            if r < c:
                mask[r] = 1.0
            elif r == c:
                mask[r] = np.tril(np.ones((TC, TC))).T
        mask = np.ascontiguousarray(mask.transpose(1, 0, 2))  # [128, NC, TC]
        lmc = lm_eff[c * VC:(c + 1) * VC]
        lmh = np.ascontiguousarray(lmc.T.reshape(DT, 128, VC)).astype(bf)
        in_maps.append({
            "x0": np.ascontiguousarray(x0h, np.float32),
            "wqk": wqk_h, "wv": wv_h, "wo": wo_h, "w1": w1_h, "w2": w2_h,
            "rp": rp, "gam": gam, "bet": bet,
            "mask": mask.astype(bf), "lmh": lmh, "onesv": ones_h,
        })
    return s_vals, in_maps


def kernel(**inputs):
    s_vals, in_maps = _prep(inputs)
    key = s_vals.tobytes()
    if key not in _CACHE:
        _CACHE[key] = _build_program(s_vals)
    nc = _CACHE[key]
    res = run_bass_kernel_spmd(nc, in_maps, list(range(NC)))
    out = np.empty((1, T, V), np.float32)
    for c in range(NC):
        out[0, :, c * VC:(c + 1) * VC] = res.results[c]["logits"].T
    return out

